# revision 35
# baseline (speedup 1.0000x reference)
"""Trainium2 Bass kernel for nn_BitwiseWavenetBlock (10-layer gated WaveNet block).

Strategy: data-parallel over batch (16 batches -> 8 cores x 2 sequential).
Each core runs the full 10-layer block on [128, 16384] activations resident in
SBUF. Dilated k=2 convs are pairs of PSUM-accumulated 128x128 matmuls against
shifted windows of an fp16 activation buffer with zeroed margins. Weight
gating (W * sigmoid(G)), per-channel scale s and biases are folded on the host.

Engine split per 1024-col chunk pair: PE 12 matmuls (N=512, weight-major so
each stationary matrix streams both chunks); ACT bias-add + fp16 cast of the
filter/gate PSUM halves (512-wide, double-buffered banks); gating multiply on
GpSimd (3 of 4 pairs) / DVE; DVE fused (psum + bias) + master accumulations
(1024-wide); 1-in-8 pairs the residual update is ACT-assisted. The residual
master lives in fp16 directly in the conv input buffer xb (updated in place);
the skip master accumulates in fp32. Each pair's skip/residual 1x1 matmuls and
master updates are deferred two pairs so the in-place xb write never races the
backward conv tap and the PE never waits on the gating chain.

Schedule trims over the original version: two alternating xb buffers so the
next batch's input load trickles in (one chunk per 5 pair-slots) across the
whole previous batch with no transition bubble; fp16 outputs (upcast on host)
to halve output DMA; layer-0 weights shipped in a separate leading DMA on the
ACT queue so the first matmul starts early.
"""

import sys

sys.path.insert(0, "/opt/trn_rl_repo")

import numpy as np

import concourse.bass as bass
import concourse.bacc as bacc
import concourse.mybir as mybir
import concourse.tile as tile
from concourse.bass_utils import run_bass_kernel_spmd

F32 = mybir.dt.float32
F16 = mybir.dt.float16
AF = mybir.ActivationFunctionType
ALU = mybir.AluOpType

N_CORES = 8
LAYERS = 10
C = 128          # channels (= partitions)
L = 16384        # sequence length
B = 16           # total batch
BPC = B // N_CORES  # batches per core (sequential)
CW = 512         # chunk width (1 PSUM bank)
MG = 256         # xb margin (>= max shift 2^8)
NH = CW // 512   # matmul halves per chunk


def _build_nc(bpc=BPC, l_len=L, layers=LAYERS, psum_bufs=2, reps=1, ew_width=None,
              mm_n=512, ew_act=None, ew_dve=None, ew_pool=None,
              wide_cast=False, act_assist_mod=8,
              input_eng="vector", late_nodefer=False, gate_dve_mod=4,
              hp_bufs=6, stgo_bufs=3, defer=2,
              dual_xb=True, out16=True, split_wts=True, trickle_stride=5,
              stg_bufs=8, warm_eng="vector", wts_eng="scalar",
              sr_wide=False, fg_wide=False, in_dma_q=1):
    if mm_n > 512 or fg_wide or sr_wide:
        bass.BassTensorEngine.MAX_MOVING_FREE_DIM_SIZE = max(mm_n, 1024)
    # ew_*: attribution-only knobs - slice that engine's ops to this many
    # columns (keeps op count/deps, removes streaming work). None = full.
    ew_act = ew_act if ew_act is not None else ew_width
    ew_dve = ew_dve if ew_dve is not None else ew_width
    ew_pool = ew_pool if ew_pool is not None else ew_width
    def _w(ap, w):
        return ap if w is None else ap[:, :w]
    def Wa(ap):
        return _w(ap, ew_act)
    def Wd(ap):
        return _w(ap, ew_dve)
    def Wp(ap):
        return _w(ap, ew_pool)
    nch = l_len // CW
    # ensure the whole next-batch input fits in the available trickle slots
    trickle_stride = max(1, min(trickle_stride, layers * (nch // 2) // nch))
    OUT_DT = F16 if out16 else F32
    nc = bacc.Bacc(None)
    x_in = nc.declare_dram_parameter("x", [bpc, C, l_len], F32, isOutput=False)
    wts_in = nc.declare_dram_parameter("wts", [C, layers * 6 * C], F16, isOutput=False)
    bias_in = nc.declare_dram_parameter("biases", [C, layers * 4], F32, isOutput=False)
    resid_out = nc.declare_dram_parameter("resid", [bpc, C, l_len], OUT_DT, isOutput=True)
    skip_out = nc.declare_dram_parameter("skip", [bpc, C, l_len], OUT_DT, isOutput=True)

    with tile.TileContext(nc) as tc:
        with (
            tc.tile_pool(name="constp", bufs=1) as constp,
            tc.tile_pool(name="masterp", bufs=1) as masterp,
            tc.tile_pool(name="workp", bufs=4) as workp,
            tc.tile_pool(name="psump", bufs=1, space="PSUM") as psump,
        ):
            wts = constp.tile([C, layers * 6 * C], F16)
            biases = constp.tile([C, layers * 4], F32)
            # weights go out on a different HWDGE queue than the input
            # chunks so neither transfer serializes behind the other
            wq = {"scalar": nc.scalar, "vector": nc.vector, "sync": nc.sync}[wts_eng]
            if split_wts:
                # layer-0 weights land first so the first matmul never waits
                # on the full weight transfer
                wq.dma_start(wts[:, 0 : 6 * C], wts_in[:, 0 : 6 * C])
                wq.dma_start(biases[:], bias_in[:])
                wq.dma_start(wts[:, 6 * C :], wts_in[:, 6 * C :])
            else:
                wq.dma_start(wts[:], wts_in[:])
                wq.dma_start(biases[:], bias_in[:])

            skip_m = masterp.tile([C, l_len], F32)
            xbs = [masterp.tile([C, l_len + 2 * MG], F16, name=f"xb{i}")
                   for i in range(2 if dual_xb else 1)]
            # zero the conv margins; extend 512 cols into the data region
            # (overwritten by the input casts) so subtile dep-tracking can't
            # miss the narrow margin overlap of boundary conv taps
            for xb_t in xbs:
                nc.vector.memset(xb_t[:, 0 : MG + 512], 0.0)
                nc.vector.memset(xb_t[:, MG + l_len - 512 : l_len + 2 * MG], 0.0)

            def wmat(l, j):
                return wts[:, (l * 6 + j) * C : (l * 6 + j + 1) * C]

            def bvec(l, j):
                return biases[:, l * 4 + j : l * 4 + j + 1]

            def emit_input(b, c_lo, c_hi, xb, eng=None):
                for c in range(c_lo, c_hi):
                    stg_in = workp.tile([C, CW], F32, tag="stg", name="stg_in",
                                        bufs=stg_bufs)
                    dq = (nc.sync, nc.scalar)[c % in_dma_q]
                    dq.dma_start(stg_in[:], x_in[b, :, c * CW : (c + 1) * CW])
                    dst = xb[:, MG + c * CW : MG + (c + 1) * CW]
                    e = eng if eng is not None else input_eng
                    if e == "rotate":
                        e = ("act", "pool", "vector")[c % 3]
                    if e == "pool":
                        nc.gpsimd.tensor_copy(Wp(dst), Wp(stg_in[:]))
                    elif e == "act":
                        nc.scalar.activation(Wa(dst), Wa(stg_in[:]),
                                             AF.Identity, bias=0.0)
                    else:
                        nc.vector.tensor_copy(Wd(dst), Wd(stg_in[:]))

            n_iters = bpc * reps
            for bi in range(n_iters):
                b = bi % bpc
                xb = xbs[bi % len(xbs)]
                xb_next = xbs[(bi + 1) % len(xbs)]
                if bi == 0:
                    emit_input(b, 0, nch, xb, eng=warm_eng)
                pend = []
                for l in range(layers):
                    s0 = 1 if l == 0 else 2 ** (l - 1)
                    s1 = 0 if l == 0 else 2 ** (l - 1)
                    def sr_phase(c0, h_a, h_b, h_full, l=l, b=b, xb=xb):
                        # skip/resid 1x1 convs + master updates over a PAIR of
                        # chunks: 1024-wide PSUM tiles keep the DVE ops wide
                        s_ps = psump.tile([C, 2 * CW], F32, tag="s", name="s_ps", bufs=1)
                        r_ps = psump.tile([C, 2 * CW], F32, tag="r", name="r_ps", bufs=1)
                        if sr_wide:
                            # one N=1024 matmul per 1x1 conv: halves the
                            # PE instruction + ldweights count; the stt
                            # consumer needs the full 1024 anyway
                            nc.tensor.matmul(s_ps[:], wmat(l, 4), h_full[:], start=True, stop=True)
                            nc.tensor.matmul(r_ps[:], wmat(l, 5), h_full[:], start=True, stop=True)
                        else:
                            for i, h_t in ((0, h_a), (1, h_b)):
                                hs = slice(i * CW, (i + 1) * CW)
                                nc.tensor.matmul(s_ps[:, hs], wmat(l, 4), h_t[:], start=True, stop=True)
                            for i, h_t in ((0, h_a), (1, h_b)):
                                hs = slice(i * CW, (i + 1) * CW)
                                nc.tensor.matmul(r_ps[:, hs], wmat(l, 5), h_t[:], start=True, stop=True)
                        cs = slice(c0 * CW, (c0 + 2) * CW)
                        xs = xb[:, MG + c0 * CW : MG + (c0 + 2) * CW]
                        final = l == layers - 1
                        if not (final and out16):
                            if l == 0:
                                # skip master uninitialized: write, don't accumulate
                                nc.vector.tensor_scalar_add(Wd(skip_m[:, cs]), Wd(s_ps[:]), bvec(l, 2))
                            else:
                                nc.vector.scalar_tensor_tensor(
                                    Wd(skip_m[:, cs]), Wd(s_ps[:]), bvec(l, 2), Wd(skip_m[:, cs]),
                                    op0=ALU.add, op1=ALU.add,
                                )
                        if final:
                            stg = workp.tile([C, 2 * CW], OUT_DT, tag="stgo", name="stg", bufs=stgo_bufs)
                            nc.vector.scalar_tensor_tensor(
                                Wd(stg[:]), Wd(r_ps[:]), bvec(l, 3), Wd(xs),
                                op0=ALU.add, op1=ALU.add,
                            )
                            nc.sync.dma_start(resid_out[b, :, cs], stg[:])
                            if out16:
                                # last skip update goes straight to an fp16
                                # staging tile (skip_m holds layers 0..l-1)
                                sstg = workp.tile([C, 2 * CW], F16, tag="sstg",
                                                  name="sstg", bufs=stgo_bufs)
                                if l == 0:
                                    nc.vector.tensor_scalar_add(Wd(sstg[:]), Wd(s_ps[:]), bvec(l, 2))
                                else:
                                    nc.vector.scalar_tensor_tensor(
                                        Wd(sstg[:]), Wd(s_ps[:]), bvec(l, 2), Wd(skip_m[:, cs]),
                                        op0=ALU.add, op1=ALU.add,
                                    )
                                nc.sync.dma_start(skip_out[b, :, cs], sstg[:])
                            else:
                                nc.sync.dma_start(skip_out[b, :, cs], skip_m[:, cs])
                        elif act_assist_mod and (c0 // 2) % act_assist_mod == 0:
                            # ACT-assisted residual update on a subset of
                            # pairs: offload PSUM read+bias to ACT, leaving
                            # DVE a cheap fp16 2x-mode add (engine rebalance)
                            r_t = workp.tile([C, 2 * CW], F16, tag="rt", name="r_t", bufs=2)
                            nc.scalar.activation(Wa(r_t[:]), Wa(r_ps[:]),
                                                 AF.Identity, bias=bvec(l, 3))
                            nc.vector.tensor_add(Wd(xs), Wd(xs), Wd(r_t[:]))
                        else:
                            # in-place fp16 residual-master update
                            nc.vector.scalar_tensor_tensor(
                                Wd(xs), Wd(r_ps[:]), bvec(l, 3), Wd(xs),
                                op0=ALU.add, op1=ALU.add,
                            )

                    for c0 in range(0, nch, 2):
                        cc = (c0, c0 + 1)
                        if mm_n == 2 * CW:
                            # single wide tile per pair; one N=1024 matmul per
                            # weight (fewer PE instructions, bufs=1)
                            f_ps = [psump.tile([C, 2 * CW], F32, tag="f", name="f_ps", bufs=1)]
                            g_ps = [psump.tile([C, 2 * CW], F32, tag="g", name="g_ps", bufs=1)]
                            spans = [(0, 2 * CW)]
                        elif wide_cast or fg_wide:
                            # one 1024-wide (2-bank) tile per conv; matmuls
                            # write 512-wide halves (or one 1024 mm with
                            # fg_wide); bias-casts read it in halves unless
                            # wide_cast
                            f_ps = [psump.tile([C, 2 * CW], F32, tag="f", name="f_ps", bufs=1)]
                            g_ps = [psump.tile([C, 2 * CW], F32, tag="g", name="g_ps", bufs=1)]
                            spans = [(0, CW), (CW, CW)]
                        else:
                            # f/g PSUM as 512-wide single-bank tiles, both
                            # chunks of the pair at once (2 bufs per tag)
                            f_ps = [psump.tile([C, CW], F32, tag="f", name="f_ps",
                                               bufs=psum_bufs) for _ in cc]
                            g_ps = [psump.tile([C, CW], F32, tag="g", name="g_ps",
                                               bufs=psum_bufs) for _ in cc]
                            spans = [(0, CW), (CW, CW)]
                        fbp = workp.tile([C, 2 * CW], F16, tag="fb", name="fbp")
                        gbp = workp.tile([C, 2 * CW], F16, tag="gb", name="gbp")
                        # weight-major across the pair: each stationary matrix
                        # streams both chunks back-to-back before switching
                        for j, sh, start in (
                            (0, -s0, True),
                            (1, s1, False),
                            (2, -s0, True),
                            (3, s1, False),
                        ):
                            ps = f_ps if j < 2 else g_ps
                            if fg_wide:
                                # one N=1024 matmul per tap (fewer PE
                                # instructions and weight loads)
                                col = c0 * CW
                                nc.tensor.matmul(
                                    ps[0][:],
                                    wmat(l, j),
                                    xb[:, MG + col + sh : MG + col + sh + 2 * CW],
                                    start=start,
                                    stop=not start,
                                )
                            else:
                                for i, (off, w) in enumerate(spans):
                                    col = c0 * CW + off
                                    tgt = ps[i][:] if not wide_cast else ps[0][:, off:off + w]
                                    nc.tensor.matmul(
                                        tgt,
                                        wmat(l, j),
                                        xb[:, MG + col + sh : MG + col + sh + w],
                                        start=start,
                                        stop=not start,
                                    )
                            if not start:
                                # both taps done: bias-cast
                                fg_b = fbp if j < 2 else gbp
                                if wide_cast:
                                    nc.scalar.activation(
                                        Wa(fg_b[:]), Wa(ps[0][:]),
                                        AF.Identity, bias=bvec(l, j // 2),
                                    )
                                else:
                                    for i, (off, w) in enumerate(spans):
                                        src = ps[i][:] if len(ps) > 1 else ps[0][:, off:off + w]
                                        nc.scalar.activation(
                                            Wa(fg_b[:, off:off + w]),
                                            Wa(src),
                                            AF.Identity, bias=bvec(l, j // 2),
                                        )
                        # deferred skip/resid phase: pair k is emitted `defer`
                        # pairs after its in-place xb write became safe, so
                        # the PE never waits on the ACT->GpSimd gating chain
                        thresh = 1 if (late_nodefer and l == layers - 1) else defer
                        while len(pend) >= thresh:
                            fn_args = pend.pop(0)
                            fn_args[0](*fn_args[1:])

                        hp = workp.tile([C, 2 * CW], F16, tag="h", name="hp", bufs=hp_bufs)
                        if gate_dve_mod and (c0 // 2) % gate_dve_mod == 0:
                            nc.vector.tensor_mul(Wd(hp[:]), Wd(fbp[:]), Wd(gbp[:]))
                        else:
                            nc.gpsimd.tensor_mul(Wp(hp[:]), Wp(fbp[:]), Wp(gbp[:]))
                        pend.append((sr_phase, c0, hp[:, 0:CW], hp[:, CW:2 * CW], hp))
                        # dual-xb: trickle the NEXT batch's input load into its
                        # own buffer, one chunk every `trickle_stride` pair
                        # slots, spread across the whole batch so the saturated
                        # engines absorb it gradually
                        if dual_xb and bi + 1 < n_iters:
                            g = l * (nch // 2) + c0 // 2
                            if g % trickle_stride == 0 and g // trickle_stride < nch:
                                ci = g // trickle_stride
                                emit_input((bi + 1) % bpc, ci, ci + 1, xb_next)
                nb = (bi + 1) % bpc
                if not dual_xb:
                    # overlap the next batch's input load with this batch's
                    # drain: chunks 0..nch-5 have no remaining readers once the
                    # pair loop is emitted; the last two pairs' flush still
                    # reads xb chunks nch-4..nch-1, so those load after
                    if bi + 1 < n_iters:
                        emit_input(nb, 0, nch - 4, xb)
                    for p in pend:
                        p[0](*p[1:])
                    if bi + 1 < n_iters:
                        emit_input(nb, nch - 4, nch, xb)
                else:
                    for p in pend:
                        p[0](*p[1:])

    nc.finalize()
    return nc


def _sigmoid(x):
    return 1.0 / (1.0 + np.exp(-x))


def _fold(W, G, b, s):
    W = np.asarray(W, np.float32)
    G = np.asarray(G, np.float32)
    b = np.asarray(b, np.float32)
    s = np.asarray(s, np.float32)
    Weff = s[:, :, None, None] * W * _sigmoid(G)
    return Weff.astype(np.float32), (s * b).astype(np.float32)


def _prep_params(Wf, Gf, bf, sf, Wg, Gg, bg, sg, Wr, Gr, br, sr, Ws, Gs, bs, ss,
                 layers=LAYERS):
    Wf_e, bf_e = _fold(Wf, Gf, bf, sf)
    Wg_e, bg_e = _fold(Wg, Gg, bg, sg)
    Wr_e, br_e = _fold(Wr, Gr, br, sr)
    Ws_e, bs_e = _fold(Ws, Gs, bs, ss)

    # wts_host[p, l*6+j, m] = lhsT_j[p, m] = W'_j[m, p] (stationary = W'^T)
    wts_host = np.zeros((C, layers * 6, C), np.float32)
    bias_host = np.zeros((C, layers * 4), np.float32)
    for l in range(layers):
        mats = [Wf_e[l, :, :, 0], Wf_e[l, :, :, 1],
                Wg_e[l, :, :, 0], Wg_e[l, :, :, 1],
                Ws_e[l, :, :, 0], Wr_e[l, :, :, 0]]
        for j, m in enumerate(mats):
            wts_host[:, l * 6 + j, :] = m.T
        bias_host[:, l * 4 + 0] = bf_e[l]
        bias_host[:, l * 4 + 1] = bg_e[l]
        bias_host[:, l * 4 + 2] = bs_e[l]
        bias_host[:, l * 4 + 3] = br_e[l]
    wts_host = wts_host.reshape(C, layers * 6 * C).astype(np.float16)
    return wts_host, bias_host


_NC_CACHE = {}


def _make_runner(nc, n_cores=N_CORES):
    """Persistent jitted multi-core runner (same machinery as the axon path of
    run_bass_kernel_spmd, but reusable across calls without recompiling)."""
    import jax
    from jax.sharding import Mesh, PartitionSpec
    from jax.experimental.shard_map import shard_map
    from concourse.bass2jax import (
        _bass_exec_p, install_neuronx_cc_hook, partition_id_tensor)

    install_neuronx_cc_hook()
    partition_name = nc.partition_id_tensor.name if nc.partition_id_tensor else None
    in_names, out_names, out_avals = [], [], []
    for alloc in nc.m.functions[0].allocations:
        if not isinstance(alloc, mybir.MemoryLocationSet):
            continue
        name = alloc.memorylocations[0].name
        if alloc.kind == "ExternalInput":
            if name != partition_name:
                in_names.append(name)
        elif alloc.kind == "ExternalOutput":
            out_names.append(name)
            out_avals.append(jax.core.ShapedArray(
                tuple(alloc.tensor_shape), mybir.dt.np(alloc.dtype)))
    n_params = len(in_names)
    all_in = list(in_names) + list(out_names)
    if partition_name is not None:
        all_in.append(partition_name)

    def _body(*args):
        operands = list(args)
        if partition_name is not None:
            operands.append(partition_id_tensor())
        outs = _bass_exec_p.bind(
            *operands,
            out_avals=tuple(out_avals), in_names=tuple(all_in),
            out_names=tuple(out_names), lowering_input_output_aliases=(),
            sim_require_finite=True, sim_require_nnan=True, nc=nc)
        return tuple(outs)

    mesh = Mesh(np.asarray(jax.devices()[:n_cores]), ("core",))
    in_specs = (PartitionSpec("core"),) * (n_params + len(out_names))
    out_specs = (PartitionSpec("core"),) * len(out_names)
    fn = jax.jit(shard_map(_body, mesh=mesh, in_specs=in_specs,
                           out_specs=out_specs, check_rep=False),
                 keep_unused=True)
    return fn, in_names, out_names, out_avals


def kernel(x, Wf, Gf, bf, sf, Wg, Gg, bg, sg, Wr, Gr, br, sr, Ws, Gs, bs, ss):
    x = np.asarray(x, np.float32)
    wts_host, bias_host = _prep_params(Wf, Gf, bf, sf, Wg, Gg, bg, sg,
                                       Wr, Gr, br, sr, Ws, Gs, bs, ss)
    if "nc" not in _NC_CACHE:
        _NC_CACHE["nc"] = _build_nc()
    nc = _NC_CACHE["nc"]

    per_core = {
        "x": np.concatenate([x[c * BPC:(c + 1) * BPC] for c in range(N_CORES)], axis=0),
        "wts": np.concatenate([wts_host] * N_CORES, axis=0),
        "biases": np.concatenate([bias_host] * N_CORES, axis=0),
    }
    if "runner" not in _NC_CACHE:
        in_maps = [
            {"x": np.ascontiguousarray(x[c * BPC : (c + 1) * BPC]),
             "wts": wts_host, "biases": bias_host}
            for c in range(N_CORES)
        ]
        res = run_bass_kernel_spmd(nc, in_maps, list(range(N_CORES)))
        resid = np.concatenate(
            [np.asarray(res.results[c]["resid"], np.float32) for c in range(N_CORES)],
            axis=0)
        skip = np.concatenate(
            [np.asarray(res.results[c]["skip"], np.float32) for c in range(N_CORES)],
            axis=0)
        _NC_CACHE["runner"] = _make_runner(nc)
        return resid, skip

    fn, in_names, out_names, out_avals = _NC_CACHE["runner"]
    args = [per_core[n] for n in in_names]
    zouts = [np.zeros((N_CORES * av.shape[0], *av.shape[1:]), av.dtype)
             for av in out_avals]
    outs = fn(*args, *zouts)
    res = {n: np.asarray(outs[i], np.float32) for i, n in enumerate(out_names)}
    resid = res["resid"].reshape(B, C, L)
    skip = res["skip"].reshape(B, C, L)
    return resid, skip



# revision 44
# speedup vs baseline: 1.0364x; 1.0364x over previous
"""Trainium2 Bass kernel for nn_BitwiseWavenetBlock (10-layer gated WaveNet block).

Strategy: data-parallel over batch (16 batches -> 8 cores x 2 sequential).
Each core runs the full 10-layer block on [128, 16384] activations resident in
SBUF. Dilated k=2 convs are pairs of PSUM-accumulated 128x128 matmuls against
shifted windows of an fp16 activation buffer with zeroed margins. Weight
gating (W * sigmoid(G)), per-channel scale s and biases are folded on the host.

Engine split per 1024-col chunk pair: PE 12 matmuls (N=512, weight-major so
each stationary matrix streams both chunks); ACT bias-add + fp16 cast of the
filter/gate PSUM halves (512-wide, double-buffered banks); gating multiply on
GpSimd (3 of 4 pairs) / DVE; DVE fused (psum + bias) + master accumulations
(1024-wide); 1-in-8 pairs the residual update is ACT-assisted. The residual
master lives in fp16 directly in the conv input buffer xb (updated in place);
the skip master accumulates in fp32. Each pair's skip/residual 1x1 matmuls and
master updates are deferred two pairs so the in-place xb write never races the
backward conv tap and the PE never waits on the gating chain.

Schedule trims over the original version: two alternating xb buffers so the
next batch's input load trickles in (one chunk per 5 pair-slots) across the
whole previous batch with no transition bubble; fp16 outputs (upcast on host)
to halve output DMA; layer-0 weights shipped in a separate leading DMA on the
ACT queue so the first matmul starts early.
"""

import sys

sys.path.insert(0, "/opt/trn_rl_repo")

import numpy as np

import concourse.bass as bass
import concourse.bacc as bacc
import concourse.mybir as mybir
import concourse.tile as tile
from concourse.bass_utils import run_bass_kernel_spmd

F32 = mybir.dt.float32
F16 = mybir.dt.float16
AF = mybir.ActivationFunctionType
ALU = mybir.AluOpType

N_CORES = 8
LAYERS = 10
C = 128          # channels (= partitions)
L = 16384        # sequence length
B = 16           # total batch
BPC = B // N_CORES  # batches per core (sequential)
CW = 512         # chunk width (1 PSUM bank)
MG = 256         # xb margin (>= max shift 2^8)
NH = CW // 512   # matmul halves per chunk


def _build_nc(bpc=BPC, l_len=L, layers=LAYERS, psum_bufs=2, reps=1, ew_width=None,
              mm_n=512, ew_act=None, ew_dve=None, ew_pool=None,
              wide_cast=False, act_assist_mod=8,
              input_eng="vector", late_nodefer=False, gate_dve_mod=4,
              hp_bufs=6, stgo_bufs=3, defer=2,
              dual_xb=True, out16=True, split_wts=True, trickle_stride=5,
              stg_bufs=8, warm_eng="vector", wts_eng="scalar",
              sr_wide=False, fg_wide=False, in_dma_q=1,
              tail_gate=0, tail_assist=0):
    if mm_n > 512 or fg_wide or sr_wide:
        bass.BassTensorEngine.MAX_MOVING_FREE_DIM_SIZE = max(mm_n, 1024)
    # ew_*: attribution-only knobs - slice that engine's ops to this many
    # columns (keeps op count/deps, removes streaming work). None = full.
    ew_act = ew_act if ew_act is not None else ew_width
    ew_dve = ew_dve if ew_dve is not None else ew_width
    ew_pool = ew_pool if ew_pool is not None else ew_width
    def _w(ap, w):
        return ap if w is None else ap[:, :w]
    def Wa(ap):
        return _w(ap, ew_act)
    def Wd(ap):
        return _w(ap, ew_dve)
    def Wp(ap):
        return _w(ap, ew_pool)
    nch = l_len // CW
    # ensure the whole next-batch input fits in the available trickle slots
    trickle_stride = max(1, min(trickle_stride, layers * (nch // 2) // nch))
    OUT_DT = F16 if out16 else F32
    nc = bacc.Bacc(None)
    x_in = nc.declare_dram_parameter("x", [bpc, C, l_len], F32, isOutput=False)
    wts_in = nc.declare_dram_parameter("wts", [C, layers * 6 * C], F16, isOutput=False)
    bias_in = nc.declare_dram_parameter("biases", [C, layers * 4], F32, isOutput=False)
    resid_out = nc.declare_dram_parameter("resid", [bpc, C, l_len], OUT_DT, isOutput=True)
    skip_out = nc.declare_dram_parameter("skip", [bpc, C, l_len], OUT_DT, isOutput=True)

    with tile.TileContext(nc) as tc:
        with (
            tc.tile_pool(name="constp", bufs=1) as constp,
            tc.tile_pool(name="masterp", bufs=1) as masterp,
            tc.tile_pool(name="workp", bufs=4) as workp,
            tc.tile_pool(name="psump", bufs=1, space="PSUM") as psump,
        ):
            wts = constp.tile([C, layers * 6 * C], F16)
            biases = constp.tile([C, layers * 4], F32)
            # weights go out on a different HWDGE queue than the input
            # chunks so neither transfer serializes behind the other
            wq = {"scalar": nc.scalar, "vector": nc.vector, "sync": nc.sync}[wts_eng]
            if split_wts:
                # layer-0 weights land first so the first matmul never waits
                # on the full weight transfer
                wq.dma_start(wts[:, 0 : 6 * C], wts_in[:, 0 : 6 * C])
                wq.dma_start(biases[:], bias_in[:])
                wq.dma_start(wts[:, 6 * C :], wts_in[:, 6 * C :])
            else:
                wq.dma_start(wts[:], wts_in[:])
                wq.dma_start(biases[:], bias_in[:])

            skip_m = masterp.tile([C, l_len], F32)
            xbs = [masterp.tile([C, l_len + 2 * MG], F16, name=f"xb{i}")
                   for i in range(2 if dual_xb else 1)]
            # zero the conv margins; extend 512 cols into the data region
            # (overwritten by the input casts) so subtile dep-tracking can't
            # miss the narrow margin overlap of boundary conv taps
            for xb_t in xbs:
                nc.vector.memset(xb_t[:, 0 : MG + 512], 0.0)
                nc.vector.memset(xb_t[:, MG + l_len - 512 : l_len + 2 * MG], 0.0)

            def wmat(l, j):
                return wts[:, (l * 6 + j) * C : (l * 6 + j + 1) * C]

            def bvec(l, j):
                return biases[:, l * 4 + j : l * 4 + j + 1]

            def emit_input(b, c_lo, c_hi, xb, eng=None):
                for c in range(c_lo, c_hi):
                    stg_in = workp.tile([C, CW], F32, tag="stg", name="stg_in",
                                        bufs=stg_bufs)
                    dq = (nc.sync, nc.scalar)[c % in_dma_q]
                    dq.dma_start(stg_in[:], x_in[b, :, c * CW : (c + 1) * CW])
                    dst = xb[:, MG + c * CW : MG + (c + 1) * CW]
                    e = eng if eng is not None else input_eng
                    if e == "rotate":
                        e = ("act", "pool", "vector")[c % 3]
                    if e == "pool":
                        nc.gpsimd.tensor_copy(Wp(dst), Wp(stg_in[:]))
                    elif e == "act":
                        nc.scalar.activation(Wa(dst), Wa(stg_in[:]),
                                             AF.Identity, bias=0.0)
                    else:
                        nc.vector.tensor_copy(Wd(dst), Wd(stg_in[:]))

            n_iters = bpc * reps
            for bi in range(n_iters):
                b = bi % bpc
                xb = xbs[bi % len(xbs)]
                xb_next = xbs[(bi + 1) % len(xbs)]
                if bi == 0:
                    emit_input(b, 0, nch, xb, eng=warm_eng)
                pend = []
                for l in range(layers):
                    s0 = 1 if l == 0 else 2 ** (l - 1)
                    s1 = 0 if l == 0 else 2 ** (l - 1)
                    def sr_phase(c0, h_a, h_b, h_full, l=l, b=b, xb=xb,
                                 last_bi=(bi == n_iters - 1)):
                        # skip/resid 1x1 convs + master updates over a PAIR of
                        # chunks: 1024-wide PSUM tiles keep the DVE ops wide
                        s_ps = psump.tile([C, 2 * CW], F32, tag="s", name="s_ps", bufs=1)
                        r_ps = psump.tile([C, 2 * CW], F32, tag="r", name="r_ps", bufs=1)
                        if sr_wide:
                            # one N=1024 matmul per 1x1 conv: halves the
                            # PE instruction + ldweights count; the stt
                            # consumer needs the full 1024 anyway
                            nc.tensor.matmul(s_ps[:], wmat(l, 4), h_full[:], start=True, stop=True)
                            nc.tensor.matmul(r_ps[:], wmat(l, 5), h_full[:], start=True, stop=True)
                        else:
                            for i, h_t in ((0, h_a), (1, h_b)):
                                hs = slice(i * CW, (i + 1) * CW)
                                nc.tensor.matmul(s_ps[:, hs], wmat(l, 4), h_t[:], start=True, stop=True)
                            for i, h_t in ((0, h_a), (1, h_b)):
                                hs = slice(i * CW, (i + 1) * CW)
                                nc.tensor.matmul(r_ps[:, hs], wmat(l, 5), h_t[:], start=True, stop=True)
                        cs = slice(c0 * CW, (c0 + 2) * CW)
                        xs = xb[:, MG + c0 * CW : MG + (c0 + 2) * CW]
                        final = l == layers - 1
                        if not (final and out16):
                            if l == 0:
                                # skip master uninitialized: write, don't accumulate
                                nc.vector.tensor_scalar_add(Wd(skip_m[:, cs]), Wd(s_ps[:]), bvec(l, 2))
                            else:
                                nc.vector.scalar_tensor_tensor(
                                    Wd(skip_m[:, cs]), Wd(s_ps[:]), bvec(l, 2), Wd(skip_m[:, cs]),
                                    op0=ALU.add, op1=ALU.add,
                                )
                        if final:
                            stg = workp.tile([C, 2 * CW], OUT_DT, tag="stgo", name="stg", bufs=stgo_bufs)
                            if tail_assist and out16 and last_bi and c0 >= nch - 2 * tail_assist:
                                # drain the run's tail faster: ACT absorbs the
                                # PSUM read while DVE handles the skip stt
                                r_t = workp.tile([C, 2 * CW], F16, tag="rt", name="r_t", bufs=2)
                                nc.scalar.activation(Wa(r_t[:]), Wa(r_ps[:]),
                                                     AF.Identity, bias=bvec(l, 3))
                                nc.vector.tensor_add(Wd(stg[:]), Wd(xs), Wd(r_t[:]))
                            else:
                                nc.vector.scalar_tensor_tensor(
                                    Wd(stg[:]), Wd(r_ps[:]), bvec(l, 3), Wd(xs),
                                    op0=ALU.add, op1=ALU.add,
                                )
                            nc.sync.dma_start(resid_out[b, :, cs], stg[:])
                            if out16:
                                # last skip update goes straight to an fp16
                                # staging tile (skip_m holds layers 0..l-1)
                                sstg = workp.tile([C, 2 * CW], F16, tag="sstg",
                                                  name="sstg", bufs=stgo_bufs)
                                if l == 0:
                                    nc.vector.tensor_scalar_add(Wd(sstg[:]), Wd(s_ps[:]), bvec(l, 2))
                                else:
                                    nc.vector.scalar_tensor_tensor(
                                        Wd(sstg[:]), Wd(s_ps[:]), bvec(l, 2), Wd(skip_m[:, cs]),
                                        op0=ALU.add, op1=ALU.add,
                                    )
                                nc.sync.dma_start(skip_out[b, :, cs], sstg[:])
                            else:
                                nc.sync.dma_start(skip_out[b, :, cs], skip_m[:, cs])
                        elif act_assist_mod and (c0 // 2) % act_assist_mod == 0:
                            # ACT-assisted residual update on a subset of
                            # pairs: offload PSUM read+bias to ACT, leaving
                            # DVE a cheap fp16 2x-mode add (engine rebalance)
                            r_t = workp.tile([C, 2 * CW], F16, tag="rt", name="r_t", bufs=2)
                            nc.scalar.activation(Wa(r_t[:]), Wa(r_ps[:]),
                                                 AF.Identity, bias=bvec(l, 3))
                            nc.vector.tensor_add(Wd(xs), Wd(xs), Wd(r_t[:]))
                        else:
                            # in-place fp16 residual-master update
                            nc.vector.scalar_tensor_tensor(
                                Wd(xs), Wd(r_ps[:]), bvec(l, 3), Wd(xs),
                                op0=ALU.add, op1=ALU.add,
                            )

                    for c0 in range(0, nch, 2):
                        cc = (c0, c0 + 1)
                        if mm_n == 2 * CW:
                            # single wide tile per pair; one N=1024 matmul per
                            # weight (fewer PE instructions, bufs=1)
                            f_ps = [psump.tile([C, 2 * CW], F32, tag="f", name="f_ps", bufs=1)]
                            g_ps = [psump.tile([C, 2 * CW], F32, tag="g", name="g_ps", bufs=1)]
                            spans = [(0, 2 * CW)]
                        elif wide_cast or fg_wide:
                            # one 1024-wide (2-bank) tile per conv; matmuls
                            # write 512-wide halves (or one 1024 mm with
                            # fg_wide); bias-casts read it in halves unless
                            # wide_cast
                            f_ps = [psump.tile([C, 2 * CW], F32, tag="f", name="f_ps", bufs=1)]
                            g_ps = [psump.tile([C, 2 * CW], F32, tag="g", name="g_ps", bufs=1)]
                            spans = [(0, CW), (CW, CW)]
                        else:
                            # f/g PSUM as 512-wide single-bank tiles, both
                            # chunks of the pair at once (2 bufs per tag)
                            f_ps = [psump.tile([C, CW], F32, tag="f", name="f_ps",
                                               bufs=psum_bufs) for _ in cc]
                            g_ps = [psump.tile([C, CW], F32, tag="g", name="g_ps",
                                               bufs=psum_bufs) for _ in cc]
                            spans = [(0, CW), (CW, CW)]
                        fbp = workp.tile([C, 2 * CW], F16, tag="fb", name="fbp")
                        gbp = workp.tile([C, 2 * CW], F16, tag="gb", name="gbp")
                        # weight-major across the pair: each stationary matrix
                        # streams both chunks back-to-back before switching
                        for j, sh, start in (
                            (0, -s0, True),
                            (1, s1, False),
                            (2, -s0, True),
                            (3, s1, False),
                        ):
                            ps = f_ps if j < 2 else g_ps
                            if fg_wide:
                                # one N=1024 matmul per tap (fewer PE
                                # instructions and weight loads)
                                col = c0 * CW
                                nc.tensor.matmul(
                                    ps[0][:],
                                    wmat(l, j),
                                    xb[:, MG + col + sh : MG + col + sh + 2 * CW],
                                    start=start,
                                    stop=not start,
                                )
                            else:
                                for i, (off, w) in enumerate(spans):
                                    col = c0 * CW + off
                                    tgt = ps[i][:] if not wide_cast else ps[0][:, off:off + w]
                                    nc.tensor.matmul(
                                        tgt,
                                        wmat(l, j),
                                        xb[:, MG + col + sh : MG + col + sh + w],
                                        start=start,
                                        stop=not start,
                                    )
                            if not start:
                                # both taps done: bias-cast
                                fg_b = fbp if j < 2 else gbp
                                if wide_cast:
                                    nc.scalar.activation(
                                        Wa(fg_b[:]), Wa(ps[0][:]),
                                        AF.Identity, bias=bvec(l, j // 2),
                                    )
                                else:
                                    for i, (off, w) in enumerate(spans):
                                        src = ps[i][:] if len(ps) > 1 else ps[0][:, off:off + w]
                                        nc.scalar.activation(
                                            Wa(fg_b[:, off:off + w]),
                                            Wa(src),
                                            AF.Identity, bias=bvec(l, j // 2),
                                        )
                        # deferred skip/resid phase: pair k is emitted `defer`
                        # pairs after its in-place xb write became safe, so
                        # the PE never waits on the ACT->GpSimd gating chain
                        in_tail = (l == layers - 1 and bi == n_iters - 1
                                   and c0 >= nch - 2 * tail_gate)
                        thresh = 1 if ((late_nodefer and l == layers - 1) or in_tail) else defer
                        while len(pend) >= thresh:
                            fn_args = pend.pop(0)
                            fn_args[0](*fn_args[1:])

                        hp = workp.tile([C, 2 * CW], F16, tag="h", name="hp", bufs=hp_bufs)
                        if in_tail or (gate_dve_mod and (c0 // 2) % gate_dve_mod == 0):
                            # tail pairs gate on DVE: shorter latency than the
                            # Pool path once the pipeline is draining
                            nc.vector.tensor_mul(Wd(hp[:]), Wd(fbp[:]), Wd(gbp[:]))
                        else:
                            nc.gpsimd.tensor_mul(Wp(hp[:]), Wp(fbp[:]), Wp(gbp[:]))
                        pend.append((sr_phase, c0, hp[:, 0:CW], hp[:, CW:2 * CW], hp))
                        # dual-xb: trickle the NEXT batch's input load into its
                        # own buffer, one chunk every `trickle_stride` pair
                        # slots, spread across the whole batch so the saturated
                        # engines absorb it gradually
                        if dual_xb and bi + 1 < n_iters:
                            g = l * (nch // 2) + c0 // 2
                            if g % trickle_stride == 0 and g // trickle_stride < nch:
                                ci = g // trickle_stride
                                emit_input((bi + 1) % bpc, ci, ci + 1, xb_next)
                nb = (bi + 1) % bpc
                if not dual_xb:
                    # overlap the next batch's input load with this batch's
                    # drain: chunks 0..nch-5 have no remaining readers once the
                    # pair loop is emitted; the last two pairs' flush still
                    # reads xb chunks nch-4..nch-1, so those load after
                    if bi + 1 < n_iters:
                        emit_input(nb, 0, nch - 4, xb)
                    for p in pend:
                        p[0](*p[1:])
                    if bi + 1 < n_iters:
                        emit_input(nb, nch - 4, nch, xb)
                else:
                    for p in pend:
                        p[0](*p[1:])

    nc.finalize()
    return nc


def _sigmoid(x):
    return 1.0 / (1.0 + np.exp(-x))


def _fold(W, G, b, s):
    W = np.asarray(W, np.float32)
    G = np.asarray(G, np.float32)
    b = np.asarray(b, np.float32)
    s = np.asarray(s, np.float32)
    Weff = s[:, :, None, None] * W * _sigmoid(G)
    return Weff.astype(np.float32), (s * b).astype(np.float32)


def _prep_params(Wf, Gf, bf, sf, Wg, Gg, bg, sg, Wr, Gr, br, sr, Ws, Gs, bs, ss,
                 layers=LAYERS):
    Wf_e, bf_e = _fold(Wf, Gf, bf, sf)
    Wg_e, bg_e = _fold(Wg, Gg, bg, sg)
    Wr_e, br_e = _fold(Wr, Gr, br, sr)
    Ws_e, bs_e = _fold(Ws, Gs, bs, ss)

    # wts_host[p, l*6+j, m] = lhsT_j[p, m] = W'_j[m, p] (stationary = W'^T)
    wts_host = np.zeros((C, layers * 6, C), np.float32)
    bias_host = np.zeros((C, layers * 4), np.float32)
    for l in range(layers):
        mats = [Wf_e[l, :, :, 0], Wf_e[l, :, :, 1],
                Wg_e[l, :, :, 0], Wg_e[l, :, :, 1],
                Ws_e[l, :, :, 0], Wr_e[l, :, :, 0]]
        for j, m in enumerate(mats):
            wts_host[:, l * 6 + j, :] = m.T
        bias_host[:, l * 4 + 0] = bf_e[l]
        bias_host[:, l * 4 + 1] = bg_e[l]
        bias_host[:, l * 4 + 2] = bs_e[l]
        bias_host[:, l * 4 + 3] = br_e[l]
    wts_host = wts_host.reshape(C, layers * 6 * C).astype(np.float16)
    return wts_host, bias_host


_NC_CACHE = {}


def _make_runner(nc, n_cores=N_CORES):
    """Persistent jitted multi-core runner (same machinery as the axon path of
    run_bass_kernel_spmd, but reusable across calls without recompiling)."""
    import jax
    from jax.sharding import Mesh, PartitionSpec
    from jax.experimental.shard_map import shard_map
    from concourse.bass2jax import (
        _bass_exec_p, install_neuronx_cc_hook, partition_id_tensor)

    install_neuronx_cc_hook()
    partition_name = nc.partition_id_tensor.name if nc.partition_id_tensor else None
    in_names, out_names, out_avals = [], [], []
    for alloc in nc.m.functions[0].allocations:
        if not isinstance(alloc, mybir.MemoryLocationSet):
            continue
        name = alloc.memorylocations[0].name
        if alloc.kind == "ExternalInput":
            if name != partition_name:
                in_names.append(name)
        elif alloc.kind == "ExternalOutput":
            out_names.append(name)
            out_avals.append(jax.core.ShapedArray(
                tuple(alloc.tensor_shape), mybir.dt.np(alloc.dtype)))
    n_params = len(in_names)
    all_in = list(in_names) + list(out_names)
    if partition_name is not None:
        all_in.append(partition_name)

    def _body(*args):
        operands = list(args)
        if partition_name is not None:
            operands.append(partition_id_tensor())
        outs = _bass_exec_p.bind(
            *operands,
            out_avals=tuple(out_avals), in_names=tuple(all_in),
            out_names=tuple(out_names), lowering_input_output_aliases=(),
            sim_require_finite=True, sim_require_nnan=True, nc=nc)
        return tuple(outs)

    mesh = Mesh(np.asarray(jax.devices()[:n_cores]), ("core",))
    in_specs = (PartitionSpec("core"),) * (n_params + len(out_names))
    out_specs = (PartitionSpec("core"),) * len(out_names)
    fn = jax.jit(shard_map(_body, mesh=mesh, in_specs=in_specs,
                           out_specs=out_specs, check_rep=False),
                 keep_unused=True)
    return fn, in_names, out_names, out_avals


def kernel(x, Wf, Gf, bf, sf, Wg, Gg, bg, sg, Wr, Gr, br, sr, Ws, Gs, bs, ss):
    x = np.asarray(x, np.float32)
    wts_host, bias_host = _prep_params(Wf, Gf, bf, sf, Wg, Gg, bg, sg,
                                       Wr, Gr, br, sr, Ws, Gs, bs, ss)
    if "nc" not in _NC_CACHE:
        _NC_CACHE["nc"] = _build_nc()
    nc = _NC_CACHE["nc"]

    per_core = {
        "x": np.concatenate([x[c * BPC:(c + 1) * BPC] for c in range(N_CORES)], axis=0),
        "wts": np.concatenate([wts_host] * N_CORES, axis=0),
        "biases": np.concatenate([bias_host] * N_CORES, axis=0),
    }
    if "runner" not in _NC_CACHE:
        in_maps = [
            {"x": np.ascontiguousarray(x[c * BPC : (c + 1) * BPC]),
             "wts": wts_host, "biases": bias_host}
            for c in range(N_CORES)
        ]
        res = run_bass_kernel_spmd(nc, in_maps, list(range(N_CORES)))
        resid = np.concatenate(
            [np.asarray(res.results[c]["resid"], np.float32) for c in range(N_CORES)],
            axis=0)
        skip = np.concatenate(
            [np.asarray(res.results[c]["skip"], np.float32) for c in range(N_CORES)],
            axis=0)
        _NC_CACHE["runner"] = _make_runner(nc)
        return resid, skip

    fn, in_names, out_names, out_avals = _NC_CACHE["runner"]
    args = [per_core[n] for n in in_names]
    zouts = [np.zeros((N_CORES * av.shape[0], *av.shape[1:]), av.dtype)
             for av in out_avals]
    outs = fn(*args, *zouts)
    res = {n: np.asarray(outs[i], np.float32) for i, n in enumerate(out_names)}
    resid = res["resid"].reshape(B, C, L)
    skip = res["skip"].reshape(B, C, L)
    return resid, skip

